# revision 41
# baseline (speedup 1.0000x reference)
"""Banded (lookahead) cross-attention on 8 Trainium2 NeuronCores.

Reference computation (B=4, T=2048, D=1024, H=16, hd=64):
    Q = query @ Wq.T + bq ; K = key_value @ Wk.T + bk ; V = key_value @ Wv.T + bv
    scores = Q K^T / sqrt(hd), masked to j <= i + lookahead
    out = softmax(scores) V, concat heads, @ Wo.T + bo
Sharding: 8 cores = (batch b = c//2) x (head-half = c%2, 8 heads each).
Host sums the two outT partials per batch and adds bo.

v4 design (ic-major phase D, dual DMA queues, bf16 output partials,
arrival-ordered prologue, ACT/PE-balanced filler placement):
  - Phase D iterates ic-major (ic outer, et inner) so E(do, ic) output
    emissions become ready after each ic completes and spread across the
    kernel instead of piling into the last quarter (v2 paid ~18us of
    epilogue idle + an 8MB output-DMA tail for that).
  - Two parallel DMA HW queues (issue rate ~613ns/descriptor is the
    prologue bottleneck): scalar queue carries wq+smalls+wk, sync queue
    carries x double-chunks [128,1024] (2KB rows) + wv + wo, both in
    first-need order.  The prologue emissions are ordered by data
    arrival (A*8, B*8, C*5) because the in-order PE queue stalls on the
    first not-ready instruction.
  - Late i-chunks are ACT-bound (exp cols/block > PE stream cycles), so
    ALL 24 ready E emissions are held as ic3-phase fillers where they
    fill the PE while ACT catches up; early phases keep only their
    deadline work.  exp on ACT: ~1 col/cycle @1.2GHz + ~260ns/instr.
  - outT partials are bf16 (host sums in f32): halves output DMA.
  - ALL softmax-div chains skip the u-copies (the normalizing mults
    read the AV psum directly; DEPTH=4 of pipeline lead covers the
    longer psum-group residency, and DVE sheds 22us of copies).  Four
    E(.,3) emissions pre-accumulate their e=0..2 matmuls under the
    final div latency (sps banks first -- they free before the pps
    slots), and tail output DMAs alternate scalar/sync queues.
    DEPTH=4 is a sharp optimum: 3 and 5 both regress to ~345us.
  - Measured matmul cost: ~175ns setup (hidden when not dependency
    gated) + 0.4ns/col stream; PE stream floor here is ~229us, ACT
    ~183us, span ~288us (~9us NEFF startup + ~8us teardown are fixed).
    Do NOT move mask mults to gpsimd (its tensor_tensor is ~10x slower
    than DVE and sits on the pt->AV path: +72us measured).
"""

import sys

for _p in ("/opt/trn_rl_repo", "/opt/pypackages"):
    if _p not in sys.path:
        sys.path.append(_p)

import numpy as np
import ml_dtypes

import concourse.bass as bass
import concourse.tile as tile
from concourse import bacc, mybir
from concourse.bass_utils import run_bass_kernel_spmd

F32 = mybir.dt.float32
BF16 = mybir.dt.bfloat16
AF = mybir.ActivationFunctionType
MUL = mybir.AluOpType.mult

B, T, D = 4, 2048, 1024
H, HD = 16, 64
H_LOC = 8                    # heads per core
E_LOC = H_LOC * HD           # 512 projected dims per core
NJB = T // 128               # 16 j-blocks
NIC = T // 512               # 4 i-chunks
NDT = D // 128               # 8 contraction tiles
NET = E_LOC // 128           # 4 e-tiles (head-pairs)
SCALE = HD ** -0.5
VW = H_LOC * (HD + 1)        # 520 v_sb layout width
VH = HD + 1                  # 65

_CACHE = {}


def _groups(L):
    """Per i-chunk: list of (jb, delta, masked); delta = first valid column
    offset inside the 512-wide chunk (0 for dense)."""
    out = []
    deltas = set()
    for ic in range(NIC):
        i0 = 512 * ic
        lst = []
        for jb in range(NJB):
            j0 = 128 * jb
            if i0 + 511 + L < j0:
                break                          # fully masked from here on
            if j0 + 127 <= i0 + L:
                lst.append((jb, 0, False))     # dense
            else:
                d = j0 - L - i0
                lst.append((jb, max(d, 0), True))
                deltas.add(d)
        out.append(lst)
    return out, sorted(deltas)


def _build(L):
    groups, deltas = _groups(L)
    dpos = {d: k for k, d in enumerate(deltas)}
    nmask = max(1, len(deltas))

    nc = bacc.Bacc("TRN2", target_bir_lowering=False, debug=False)
    xqT = nc.dram_tensor("xqT", [D, T], BF16, kind="ExternalInput").ap()
    xkvT = nc.dram_tensor("xkvT", [D, T], BF16, kind="ExternalInput").ap()
    wqT = nc.dram_tensor("wqT", [D, E_LOC], BF16, kind="ExternalInput").ap()
    wkT = nc.dram_tensor("wkT", [D, E_LOC], BF16, kind="ExternalInput").ap()
    wvT = nc.dram_tensor("wvT", [D, E_LOC], BF16, kind="ExternalInput").ap()
    woT = nc.dram_tensor("woT", [E_LOC, D], BF16, kind="ExternalInput").ap()
    bq4 = nc.dram_tensor("bq4", [128, NET], F32, kind="ExternalInput").ap()
    bk4 = nc.dram_tensor("bk4", [128, NET], F32, kind="ExternalInput").ap()
    bv_row = nc.dram_tensor("bv_row", [1, E_LOC], F32, kind="ExternalInput").ap()
    masks = nc.dram_tensor("masks", [128, nmask * 512], BF16,
                           kind="ExternalInput").ap()
    ones2d = nc.dram_tensor("ones2d", [2, 128], F32, kind="ExternalInput").ap()
    outT = nc.dram_tensor("outT", [D, T], BF16, kind="ExternalOutput").ap()

    with tile.TileContext(nc) as tc:
        with tc.tile_pool(name="small", bufs=1) as small, \
             tc.tile_pool(name="persist", bufs=1) as persist, \
             tc.tile_pool(name="slabs", bufs=1) as slabs, \
             tc.tile_pool(name="ptp", bufs=7) as pt_pool, \
             tc.tile_pool(name="dv", bufs=2) as dv_pool, \
             tc.tile_pool(name="os", bufs=4) as os_pool, \
             tc.tile_pool(name="pps", bufs=2, space="PSUM") as pps, \
             tc.tile_pool(name="sps", bufs=2, space="PSUM") as sps, \
             tc.tile_pool(name="ops", bufs=2, space="PSUM") as ops:

            wq_sb = [slabs.tile([128, E_LOC], BF16, tag=f"wq{d}", name=f"wq{d}")
                     for d in range(NDT)]
            wk_sb = [slabs.tile([128, E_LOC], BF16, tag=f"wk{d}", name=f"wk{d}")
                     for d in range(NDT)]
            wv_sb = [slabs.tile([128, E_LOC], BF16, tag=f"wv{d}", name=f"wv{d}")
                     for d in range(NDT)]
            wo_sb = [slabs.tile([128, D], BF16, tag=f"wo{e}", name=f"wo{e}")
                     for e in range(NET)]
            # x as [128, 1024] double-chunks keyed (d, t//2): 2KB rows
            xq_sb = {}
            xkv_sb = {}
            for h in range(NIC // 2):
                for d in range(NDT):
                    xq_sb[(d, h)] = slabs.tile(
                        [128, 1024], BF16, tag=f"xq{d}_{h}", name=f"xq{d}_{h}")
                    xkv_sb[(d, h)] = slabs.tile(
                        [128, 1024], BF16, tag=f"xkv{d}_{h}", name=f"xkv{d}_{h}")
            bq_sb = small.tile([128, NET], F32, tag="bq")
            bk_sb = small.tile([128, NET], F32, tag="bk")
            bv_sb = small.tile([1, E_LOC], F32, tag="bv")
            bv_bc = small.tile([128, E_LOC], F32, tag="bvb")
            mk_sb = persist.tile([128, nmask * 512], BF16, tag="mk")
            ones2 = small.tile([32, 128], F32, tag="ones2")

            # ---- DMA issue: FOUR HW queues in parallel (per-queue packet
            # rate ~120-200GB/s is the prologue bottleneck).  The tensor
            # engine is data-starved until ~11us anyway, so its queue is
            # free for the early xkv chunks.
            # scalar: wq (first A) + smalls
            nc.scalar.dma_start(wq_sb[0][:], wqT[0:128, :])
            nc.scalar.dma_start(bq_sb[:], bq4[:])
            for d in range(1, NDT):
                nc.scalar.dma_start(wq_sb[d][:], wqT[128 * d:128 * (d + 1), :])
            nc.scalar.dma_start(bk_sb[:], bk4[:])
            nc.scalar.dma_start(bv_sb[:], bv_row[:])
            nc.scalar.dma_start(mk_sb[:], masks[:])
            nc.scalar.dma_start(ones2[0:2, :], ones2d[:])
            # scalar (cont.): wk after wq
            for d in range(NDT):
                nc.scalar.dma_start(wk_sb[d][:], wkT[128 * d:128 * (d + 1), :])
            # sync: xq then xkv first halves, wv, then second halves, wo
            for d in range(NDT):
                nc.sync.dma_start(xq_sb[(d, 0)][:],
                                  xqT[128 * d:128 * (d + 1), 0:1024])
            for d in range(NDT):
                nc.sync.dma_start(xkv_sb[(d, 0)][:],
                                  xkvT[128 * d:128 * (d + 1), 0:1024])
            for d in range(NDT):
                nc.sync.dma_start(wv_sb[d][:], wvT[128 * d:128 * (d + 1), :])
            for d in range(NDT):
                nc.sync.dma_start(xkv_sb[(d, 1)][:],
                                  xkvT[128 * d:128 * (d + 1), 1024:2048])
            for d in range(NDT):
                nc.sync.dma_start(xq_sb[(d, 1)][:],
                                  xqT[128 * d:128 * (d + 1), 1024:2048])
            for e in range(NET):
                nc.sync.dma_start(wo_sb[e][:], woT[128 * e:128 * (e + 1), :])

            qT = [persist.tile([128, T], BF16, tag=f"qt{i}", name=f"qt{i}")
                  for i in range(NET)]
            kT = [persist.tile([128, T], BF16, tag=f"kt{i}", name=f"kt{i}")
                  for i in range(NET)]
            v_sb = [persist.tile([128, VW], BF16, tag=f"v{i}", name=f"v{i}")
                    for i in range(NJB)]
            aT = [persist.tile([128, T], BF16, tag=f"at{i}", name=f"at{i}")
                  for i in range(NET)]

            # v ones columns (softmax denominator accumulators) written
            # once; bv broadcast across partitions for C's fused bias add
            nc.gpsimd.partition_broadcast(bv_bc[:], bv_sb[:])
            for tt in range(NJB):
                vv = v_sb[tt][:].rearrange("p (h w) -> p h w", w=VH)
                nc.gpsimd.memset(vv[:, :, HD:VH], 1.0)

            def xq_c(d, t):
                h, o = t // 2, 512 * (t % 2)
                return xq_sb[(d, h)][:, o:o + 512]

            def xkv_c(d, t):
                h, o = t // 2, 512 * (t % 2)
                return xkv_sb[(d, h)][:, o:o + 512]

            # ---- filler bundle emitters (projections / output) ----
            def emit_A(et, t):
                ps = pps.tile([128, 512], F32, tag="pp")
                for d in range(NDT):
                    nc.tensor.matmul(
                        ps[:], wq_sb[d][:, 128 * et:128 * (et + 1)],
                        xq_c(d, t), start=(d == 0), stop=(d == NDT - 1))
                nc.vector.tensor_scalar_add(
                    qT[et][:, 512 * t:512 * (t + 1)], ps[:],
                    bq_sb[:, et:et + 1])

            def emit_B(et, t):
                ps = pps.tile([128, 512], F32, tag="pp")
                for d in range(NDT):
                    nc.tensor.matmul(
                        ps[:], wk_sb[d][:, 128 * et:128 * (et + 1)],
                        xkv_c(d, t), start=(d == 0), stop=(d == NDT - 1))
                nc.vector.tensor_scalar_add(
                    kT[et][:, 512 * t:512 * (t + 1)], ps[:],
                    bk_sb[:, et:et + 1])

            def emit_C(tt):
                ps = pps.tile([128, 512], F32, tag="pp")
                t, q = tt // 4, tt % 4
                for d in range(NDT):
                    nc.tensor.matmul(
                        ps[:], xkv_c(d, t)[:, 128 * q:128 * (q + 1)],
                        wv_sb[d][:], start=(d == 0), stop=(d == NDT - 1))
                vv = v_sb[tt][:].rearrange("p (h w) -> p h w", w=VH)
                nc.vector.tensor_tensor(
                    vv[:, :, 0:HD],
                    ps[:].rearrange("p (h w) -> p h w", w=HD),
                    bv_bc[:].rearrange("p (h w) -> p h w", w=HD),
                    mybir.AluOpType.add)

            def emit_E(do, ic):
                ps = pps.tile([128, 512], F32, tag="pp")
                for e in range(NET):
                    nc.tensor.matmul(
                        ps[:], wo_sb[e][:, 128 * do:128 * (do + 1)],
                        aT[e][:, 512 * ic:512 * (ic + 1)],
                        start=(e == 0), stop=(e == NET - 1))
                o = os_pool.tile([128, 512], BF16, tag="eo")
                nc.vector.tensor_scalar_add(o[:], ps[:], 0.0)
                nc.sync.dma_start(
                    outT[128 * do:128 * (do + 1), 512 * ic:512 * (ic + 1)],
                    o[:])

            def run_filler(f):
                kind = f[0]
                if kind == "A":
                    emit_A(f[1], f[2])
                elif kind == "B":
                    emit_B(f[1], f[2])
                elif kind == "C":
                    emit_C(f[1])
                else:
                    emit_E(f[1], f[2])

            # warmup: dummy matmuls on the first-arriving wq slab ramp
            # the PE p-state (full clock needs ~3us of continuous issue)
            # before the real prologue work, filling startup idle
            wu = pps.tile([128, 512], F32, tag="pp", name="warmup")
            for r in range(6):
                nc.tensor.matmul(wu[:], wq_sb[0][:, 0:128],
                                 wq_sb[0][:, 0:512],
                                 start=(r == 0), stop=(r == 5))
            # ---- prologue, ordered by DMA arrival (in-order PE queue:
            # an emission whose data is late stalls everything behind it).
            # wq+xq01 work first, then wk+xkv01, then wv (C's) last.
            for et in range(NET):
                emit_A(et, 0)
            for et in range(NET):
                emit_A(et, 1)
            for et in range(NET):
                emit_B(et, 0)
            for et in range(NET):
                emit_B(et, 1)
            for tt in range(5):
                emit_C(tt)

            # ---- phase D, ic-major, with interleaved fillers ----
            work = []   # (et, ic, jb, dlt, msk, first, last)
            for ic in range(NIC):
                lst = groups[ic]
                for et in range(NET):
                    for (jb, dlt, msk) in lst:
                        work.append((et, ic, jb, dlt, msk,
                                     jb == lst[0][0], jb == lst[-1][0]))

            # fillers per ic-phase, in first-need order.  All E emissions
            # are deferred to the ic3 phase: ic0..ic2 are PE-bound while
            # ic3 is ACT-bound (exp cols/block exceed PE stream cycles),
            # so E inventory there keeps the PE busy while ACT catches up.
            fillers = {
                # during D(.,0): v-blocks for ic1 and ic2 (ic0 is
                # slightly ACT-bound after the prologue restructure, so
                # give it the extra PE work; xkv23 lands in time for C9+)
                0: [("C", 5), ("C", 6), ("C", 7), ("C", 8),
                    ("C", 9), ("C", 10), ("C", 11), ("C", 12)],
                # during D(.,1): ic2 prep
                1: [("B", 0, 2), ("B", 1, 2), ("B", 2, 2), ("B", 3, 2),
                    ("A", 0, 2), ("A", 1, 2), ("A", 2, 2), ("A", 3, 2)],
                # during D(.,2): ic3 prep
                2: [("B", 0, 3), ("B", 1, 3), ("B", 2, 3), ("B", 3, 3),
                    ("A", 0, 3), ("A", 1, 3), ("A", 2, 3), ("A", 3, 3)]
                   + [("C", 13), ("C", 14), ("C", 15)],
                # during D(.,3): all ready E's
                3: [("E", do, 0) for do in range(NDT)]
                   + [("E", do, 1) for do in range(NDT)]
                   + [("E", do, 2) for do in range(NDT)],
            }

            ot = {}           # (et, ic) -> (otA, otB)
            pending = {}      # n -> pt tile
            queue = []
            DEPTH = 4

            def emit_div(et, ic, fast=False):
                otA, otB = ot.pop((et, ic))
                if fast:
                    # short-latency chain, no u-copies (mults read psum
                    # directly).  Interleave (d, recip) per half so each
                    # gpsimd broadcast starts as early as possible.
                    rbs = []
                    for o in (otA, otB):
                        dt = dv_pool.tile([1, 512], F32, tag="d")
                        nc.vector.tensor_scalar_add(dt[:], o[64:65, :], 0.0)
                        r = dv_pool.tile([1, 512], F32, tag="r")
                        nc.vector.reciprocal_approx_fast(r[:], dt[:])
                        rb = dv_pool.tile([64, 512], F32, tag="rb")
                        nc.gpsimd.partition_broadcast(rb[:], r[:])
                        rbs.append(rb)
                    for half, (o, rb) in enumerate(zip((otA, otB), rbs)):
                        nc.vector.tensor_tensor(
                            aT[et][64 * half:64 * half + 64,
                                   512 * ic:512 * (ic + 1)],
                            o[0:64, :], rb[:], MUL)
                    return
                us, ds = [], []
                # two copies free the psum slot fast; the denominator row
                # goes to a base-partition-0 tile (reciprocal_approx_fast
                # mishandles nonzero base partitions)
                for o in (otA, otB):
                    u = dv_pool.tile([64, 512], F32, tag="u")
                    nc.vector.tensor_scalar_add(u[:], o[0:64, :], 0.0)
                    d = dv_pool.tile([1, 512], F32, tag="d")
                    nc.vector.tensor_scalar_add(d[:], o[64:65, :], 0.0)
                    us.append(u)
                    ds.append(d)
                rs = []
                for d in ds:
                    r = dv_pool.tile([1, 512], F32, tag="r")
                    nc.vector.reciprocal_approx_fast(r[:], d[:])
                    rs.append(r)
                rbs = []
                for r in rs:
                    rb = dv_pool.tile([64, 512], F32, tag="rb")
                    nc.gpsimd.partition_broadcast(rb[:], r[:])
                    rbs.append(rb)
                for half, (u, rb) in enumerate(zip(us, rbs)):
                    nc.vector.tensor_tensor(
                        aT[et][64 * half:64 * half + 64,
                               512 * ic:512 * (ic + 1)],
                        u[:], rb[:], MUL)

            def emit_stage2(n):
                et, ic, jb, dlt, msk, first, last = work[n]
                pt = pending.pop(n)
                if first:
                    ot[(et, ic)] = (
                        ops.tile([65, 512], F32, tag="ot", name=f"oA{et}_{ic}"),
                        ops.tile([65, 512], F32, tag="ot", name=f"oB{et}_{ic}"))
                otA, otB = ot[(et, ic)]
                hA, hB = 2 * et, 2 * et + 1
                nc.tensor.matmul(
                    otA[:, dlt:512], v_sb[jb][:, VH * hA:VH * hA + VH],
                    pt[:, dlt:512], start=first, stop=last,
                    skip_group_check=True)
                nc.tensor.matmul(
                    otB[:, dlt:512], v_sb[jb][:, VH * hB:VH * hB + VH],
                    pt[:, 512 + dlt:1024], start=first, stop=last,
                    skip_group_check=True)
                if last:
                    emit_div(et, ic, fast=True)

            # per-phase adaptive filler pacing
            ic_of = [w[1] for w in work]
            phase_len = [sum(1 for x in ic_of if x == ic) for ic in range(NIC)]
            acc = 0.0
            pos_in_phase = 0
            cur_ic = -1
            for n, (et, ic, jb, dlt, msk, first, last) in enumerate(work):
                if ic != cur_ic:
                    cur_ic = ic
                    pos_in_phase = 0
                    queue.extend(fillers[ic])
                g = pos_in_phase
                pos_in_phase += 1
                acc += len(queue) / max(1, phase_len[ic] - g)
                while acc >= 1.0 and queue:
                    run_filler(queue.pop(0))
                    acc -= 1.0

                st = sps.tile([128, 1024], F32, tag="st")
                nc.tensor.matmul(
                    st[:, dlt:512],
                    kT[et][0:64, 128 * jb:128 * (jb + 1)],
                    qT[et][0:64, 512 * ic + dlt:512 * (ic + 1)],
                    start=True, stop=True)
                nc.tensor.matmul(
                    st[:, 512 + dlt:1024],
                    kT[et][64:128, 128 * jb:128 * (jb + 1)],
                    qT[et][64:128, 512 * ic + dlt:512 * (ic + 1)],
                    start=True, stop=True)
                pt = pt_pool.tile([128, 1024], BF16, tag="pt")
                nc.scalar.activation(pt[:, dlt:1024], st[:, dlt:1024],
                                     AF.Exp, scale=SCALE)
                if msk:
                    k = dpos[128 * jb - L - 512 * ic]
                    w = min(dlt + 128, 512) - dlt
                    for off in (0, 512):
                        nc.vector.tensor_tensor(
                            pt[:, off + dlt:off + dlt + w],
                            pt[:, off + dlt:off + dlt + w],
                            mk_sb[:, 512 * k + dlt:512 * k + dlt + w], MUL)
                pending[n] = pt
                if n >= DEPTH:
                    emit_stage2(n - DEPTH)
            for n in range(max(0, len(work) - DEPTH), len(work)):
                emit_stage2(n)
            while queue:
                run_filler(queue.pop(0))
            # tail: E(.,3).  Four emissions pre-accumulate their e=0..2
            # matmuls (deps: aT[0..2] only) overlapping the final div
            # chain's latency -- two in pps slots, two borrowing the idle
            # sps (st) banks.  Tail output DMAs go out the scalar queue
            # (idle after the last exp) so the final drain isn't gated on
            # a single queue's packet rate.
            part = []
            for do in range(4):
                # sps slots first: they free as soon as the last exp is
                # read, while pps slots are held by the last E-fillers'
                # casts for ~1-2us more
                if do < 2:
                    ps = sps.tile([128, 1024], F32, tag="st",
                                  name=f"ep{do}")[:, 0:512]
                else:
                    ps = pps.tile([128, 512], F32, tag="pp",
                                  name=f"ep{do}")
                for e in range(NET - 1):
                    nc.tensor.matmul(
                        ps, wo_sb[e][:, 128 * do:128 * (do + 1)],
                        aT[e][:, 512 * 3:512 * 4],
                        start=(e == 0), stop=False)
                part.append(ps)
            for do in range(4):
                ps = part[do]
                nc.tensor.matmul(
                    ps, wo_sb[NET - 1][:, 128 * do:128 * (do + 1)],
                    aT[NET - 1][:, 512 * 3:512 * 4],
                    start=False, stop=True)
                o = os_pool.tile([128, 512], BF16, tag="eo")
                nc.vector.tensor_scalar_add(o[:], ps, 0.0)
                eng = nc.scalar if do % 2 == 0 else nc.sync
                eng.dma_start(
                    outT[128 * do:128 * (do + 1), 512 * 3:512 * 4], o[:])
            for do in range(4, NDT):
                ps = pps.tile([128, 512], F32, tag="pp")
                for e in range(NET):
                    nc.tensor.matmul(
                        ps[:], wo_sb[e][:, 128 * do:128 * (do + 1)],
                        aT[e][:, 512 * 3:512 * 4],
                        start=(e == 0), stop=(e == NET - 1))
                o = os_pool.tile([128, 512], BF16, tag="eo")
                nc.vector.tensor_scalar_add(o[:], ps[:], 0.0)
                eng = nc.scalar if do % 2 == 0 else nc.sync
                eng.dma_start(
                    outT[128 * do:128 * (do + 1), 512 * 3:512 * 4], o[:])

    nc.compile()
    return nc, deltas


def _prep_core(query, key_value, Wq, bq, Wk, bk, Wv, bv, Wo, c, deltas, L):
    b, half = c // 2, c % 2
    hs = E_LOC * half
    f32, bf16 = np.float32, ml_dtypes.bfloat16
    xqT = np.ascontiguousarray(query[b].T).astype(bf16)
    xkvT = np.ascontiguousarray(key_value[b].T).astype(bf16)
    wqT = np.ascontiguousarray(Wq[hs:hs + E_LOC].T).astype(bf16)
    wkT = np.ascontiguousarray(Wk[hs:hs + E_LOC].T).astype(bf16)
    wvT = np.ascontiguousarray(Wv[hs:hs + E_LOC].T).astype(bf16)
    bv_row = bv[hs:hs + E_LOC].reshape(1, E_LOC).astype(f32)
    woT = np.ascontiguousarray(Wo[:, hs:hs + E_LOC].T).astype(bf16)
    bq4 = np.ascontiguousarray(bq[hs:hs + E_LOC].reshape(NET, 128).T, dtype=f32)
    bk4 = np.ascontiguousarray(bk[hs:hs + E_LOC].reshape(NET, 128).T, dtype=f32)
    nmask = max(1, len(deltas))
    masks = np.zeros((128, nmask * 512), dtype=bf16)
    jr = np.arange(128)[:, None]
    ir = np.arange(512)[None, :]
    for k, d in enumerate(deltas):
        masks[:, 512 * k:512 * (k + 1)] = (jr <= ir - d).astype(bf16)
    ones2 = np.zeros((2, 128), dtype=f32)
    ones2[0, 0:64] = 1.0
    ones2[1, 64:128] = 1.0
    return {"xqT": xqT, "xkvT": xkvT, "wqT": wqT, "wkT": wkT, "wvT": wvT,
            "woT": woT, "bq4": bq4, "bk4": bk4, "bv_row": bv_row,
            "masks": masks, "ones2d": ones2}


def kernel(query, key_value, Wq, bq, Wk, bk, Wv, bv, Wo, bo, lookahead,
           _trace=False):
    L = int(lookahead)
    if L not in _CACHE:
        _CACHE[L] = _build(L)
    nc, deltas = _CACHE[L]

    args = [np.asarray(a, dtype=np.float32) for a in
            (query, key_value, Wq, bq, Wk, bk, Wv, bv, Wo)]
    in_maps = [_prep_core(*args, c, deltas, L) for c in range(8)]
    res = run_bass_kernel_spmd(nc, in_maps, core_ids=list(range(8)),
                               trace=_trace)
    bo = np.asarray(bo, dtype=np.float32)
    out = np.empty((B, T, D), dtype=np.float32)
    for b in range(B):
        pT = (res.results[2 * b]["outT"].astype(np.float32)
              + res.results[2 * b + 1]["outT"].astype(np.float32))
        out[b] = pT.T + bo[None, :]
    if _trace:
        kernel.last_exec_time_ns = res.exec_time_ns
    return out


# revision 42
# speedup vs baseline: 1.0148x; 1.0148x over previous
"""Banded (lookahead) cross-attention on 8 Trainium2 NeuronCores.

Reference computation (B=4, T=2048, D=1024, H=16, hd=64):
    Q = query @ Wq.T + bq ; K = key_value @ Wk.T + bk ; V = key_value @ Wv.T + bv
    scores = Q K^T / sqrt(hd), masked to j <= i + lookahead
    out = softmax(scores) V, concat heads, @ Wo.T + bo
Sharding: 8 cores = (batch b = c//2) x (head-half = c%2, 8 heads each).
Host sums the two outT partials per batch and adds bo.

v4 design (ic-major phase D, dual DMA queues, bf16 output partials,
arrival-ordered prologue, ACT/PE-balanced filler placement):
  - Phase D iterates ic-major (ic outer, et inner) so E(do, ic) output
    emissions become ready after each ic completes and spread across the
    kernel instead of piling into the last quarter (v2 paid ~18us of
    epilogue idle + an 8MB output-DMA tail for that).
  - Two parallel DMA HW queues (issue rate ~613ns/descriptor is the
    prologue bottleneck): scalar queue carries wq+smalls+wk, sync queue
    carries x double-chunks [128,1024] (2KB rows) + wv + wo, both in
    first-need order.  The prologue emissions are ordered by data
    arrival (A*8, B*8, C*5) because the in-order PE queue stalls on the
    first not-ready instruction.
  - Late i-chunks are ACT-bound (exp cols/block > PE stream cycles), so
    ALL 24 ready E emissions are held as ic3-phase fillers where they
    fill the PE while ACT catches up; early phases keep only their
    deadline work.  exp on ACT: ~1 col/cycle @1.2GHz + ~260ns/instr.
  - outT partials are bf16 (host sums in f32): halves output DMA.
  - ALL softmax-div chains skip the u-copies (the normalizing mults
    read the AV psum directly; DEPTH=4 of pipeline lead covers the
    longer psum-group residency, and DVE sheds 22us of copies).  Four
    E(.,3) emissions pre-accumulate their e=0..2 matmuls under the
    final div latency (sps banks first -- they free before the pps
    slots), and tail output DMAs alternate scalar/sync queues.
    DEPTH=4 is a sharp optimum: 3 and 5 both regress to ~345us.
  - Measured matmul cost: ~175ns setup (hidden when not dependency
    gated) + 0.4ns/col stream; PE stream floor here is ~229us, ACT
    ~183us, span ~288us (~9us NEFF startup + ~8us teardown are fixed).
    Do NOT move mask mults to gpsimd (its tensor_tensor is ~10x slower
    than DVE and sits on the pt->AV path: +72us measured).
"""

import sys

for _p in ("/opt/trn_rl_repo", "/opt/pypackages"):
    if _p not in sys.path:
        sys.path.append(_p)

import numpy as np
import ml_dtypes

import concourse.bass as bass
import concourse.tile as tile
from concourse import bacc, mybir
from concourse.bass_utils import run_bass_kernel_spmd

F32 = mybir.dt.float32
BF16 = mybir.dt.bfloat16
AF = mybir.ActivationFunctionType
MUL = mybir.AluOpType.mult

B, T, D = 4, 2048, 1024
H, HD = 16, 64
H_LOC = 8                    # heads per core
E_LOC = H_LOC * HD           # 512 projected dims per core
NJB = T // 128               # 16 j-blocks
NIC = T // 512               # 4 i-chunks
NDT = D // 128               # 8 contraction tiles
NET = E_LOC // 128           # 4 e-tiles (head-pairs)
SCALE = HD ** -0.5
VW = H_LOC * (HD + 1)        # 520 v_sb layout width
VH = HD + 1                  # 65

_CACHE = {}


def _groups(L):
    """Per i-chunk: list of (jb, delta, masked); delta = first valid column
    offset inside the 512-wide chunk (0 for dense)."""
    out = []
    deltas = set()
    for ic in range(NIC):
        i0 = 512 * ic
        lst = []
        for jb in range(NJB):
            j0 = 128 * jb
            if i0 + 511 + L < j0:
                break                          # fully masked from here on
            if j0 + 127 <= i0 + L:
                lst.append((jb, 0, False))     # dense
            else:
                d = j0 - L - i0
                lst.append((jb, max(d, 0), True))
                deltas.add(d)
        out.append(lst)
    return out, sorted(deltas)


def _build(L):
    groups, deltas = _groups(L)
    dpos = {d: k for k, d in enumerate(deltas)}
    nmask = max(1, len(deltas))

    nc = bacc.Bacc("TRN2", target_bir_lowering=False, debug=False)
    xqT = nc.dram_tensor("xqT", [D, T], BF16, kind="ExternalInput").ap()
    xkvT = nc.dram_tensor("xkvT", [D, T], BF16, kind="ExternalInput").ap()
    wqT = nc.dram_tensor("wqT", [D, E_LOC], BF16, kind="ExternalInput").ap()
    wkT = nc.dram_tensor("wkT", [D, E_LOC], BF16, kind="ExternalInput").ap()
    wvT = nc.dram_tensor("wvT", [D, E_LOC], BF16, kind="ExternalInput").ap()
    woT = nc.dram_tensor("woT", [E_LOC, D], BF16, kind="ExternalInput").ap()
    bq4 = nc.dram_tensor("bq4", [128, NET], F32, kind="ExternalInput").ap()
    bk4 = nc.dram_tensor("bk4", [128, NET], F32, kind="ExternalInput").ap()
    bv_row = nc.dram_tensor("bv_row", [1, E_LOC], F32, kind="ExternalInput").ap()
    masks = nc.dram_tensor("masks", [128, nmask * 512], BF16,
                           kind="ExternalInput").ap()
    ones2d = nc.dram_tensor("ones2d", [2, 128], F32, kind="ExternalInput").ap()
    outT = nc.dram_tensor("outT", [D, T], BF16, kind="ExternalOutput").ap()

    with tile.TileContext(nc) as tc:
        with tc.tile_pool(name="small", bufs=1) as small, \
             tc.tile_pool(name="persist", bufs=1) as persist, \
             tc.tile_pool(name="slabs", bufs=1) as slabs, \
             tc.tile_pool(name="ptp", bufs=7) as pt_pool, \
             tc.tile_pool(name="dv", bufs=2) as dv_pool, \
             tc.tile_pool(name="os", bufs=4) as os_pool, \
             tc.tile_pool(name="pps", bufs=2, space="PSUM") as pps, \
             tc.tile_pool(name="sps", bufs=2, space="PSUM") as sps, \
             tc.tile_pool(name="ops", bufs=2, space="PSUM") as ops:

            wq_sb = [slabs.tile([128, E_LOC], BF16, tag=f"wq{d}", name=f"wq{d}")
                     for d in range(NDT)]
            wk_sb = [slabs.tile([128, E_LOC], BF16, tag=f"wk{d}", name=f"wk{d}")
                     for d in range(NDT)]
            wv_sb = [slabs.tile([128, E_LOC], BF16, tag=f"wv{d}", name=f"wv{d}")
                     for d in range(NDT)]
            wo_sb = [slabs.tile([128, D], BF16, tag=f"wo{e}", name=f"wo{e}")
                     for e in range(NET)]
            # x as [128, 1024] double-chunks keyed (d, t//2): 2KB rows
            xq_sb = {}
            xkv_sb = {}
            for h in range(NIC // 2):
                for d in range(NDT):
                    xq_sb[(d, h)] = slabs.tile(
                        [128, 1024], BF16, tag=f"xq{d}_{h}", name=f"xq{d}_{h}")
                    xkv_sb[(d, h)] = slabs.tile(
                        [128, 1024], BF16, tag=f"xkv{d}_{h}", name=f"xkv{d}_{h}")
            bq_sb = small.tile([128, NET], F32, tag="bq")
            bk_sb = small.tile([128, NET], F32, tag="bk")
            bv_sb = small.tile([1, E_LOC], F32, tag="bv")
            bv_bc = small.tile([128, E_LOC], F32, tag="bvb")
            mk_sb = persist.tile([128, nmask * 512], BF16, tag="mk")
            ones2 = small.tile([32, 128], F32, tag="ones2")

            # ---- DMA issue: FOUR HW queues in parallel (per-queue packet
            # rate ~120-200GB/s is the prologue bottleneck).  The tensor
            # engine is data-starved until ~11us anyway, so its queue is
            # free for the early xkv chunks.
            # scalar: wq (first A) + smalls
            nc.scalar.dma_start(wq_sb[0][:], wqT[0:128, :])
            nc.scalar.dma_start(bq_sb[:], bq4[:])
            for d in range(1, NDT):
                nc.scalar.dma_start(wq_sb[d][:], wqT[128 * d:128 * (d + 1), :])
            nc.scalar.dma_start(bk_sb[:], bk4[:])
            nc.scalar.dma_start(bv_sb[:], bv_row[:])
            nc.scalar.dma_start(mk_sb[:], masks[:])
            nc.scalar.dma_start(ones2[0:2, :], ones2d[:])
            # scalar (cont.): wk after wq
            for d in range(NDT):
                nc.scalar.dma_start(wk_sb[d][:], wkT[128 * d:128 * (d + 1), :])
            # sync: xq then xkv first halves, wv, then second halves, wo
            for d in range(NDT):
                nc.sync.dma_start(xq_sb[(d, 0)][:],
                                  xqT[128 * d:128 * (d + 1), 0:1024])
            for d in range(NDT):
                nc.sync.dma_start(xkv_sb[(d, 0)][:],
                                  xkvT[128 * d:128 * (d + 1), 0:1024])
            for d in range(NDT):
                nc.sync.dma_start(wv_sb[d][:], wvT[128 * d:128 * (d + 1), :])
            for d in range(NDT):
                nc.sync.dma_start(xkv_sb[(d, 1)][:],
                                  xkvT[128 * d:128 * (d + 1), 1024:2048])
            for d in range(NDT):
                nc.sync.dma_start(xq_sb[(d, 1)][:],
                                  xqT[128 * d:128 * (d + 1), 1024:2048])
            for e in range(NET):
                nc.sync.dma_start(wo_sb[e][:], woT[128 * e:128 * (e + 1), :])

            qT = [persist.tile([128, T], BF16, tag=f"qt{i}", name=f"qt{i}")
                  for i in range(NET)]
            kT = [persist.tile([128, T], BF16, tag=f"kt{i}", name=f"kt{i}")
                  for i in range(NET)]
            v_sb = [persist.tile([128, VW], BF16, tag=f"v{i}", name=f"v{i}")
                    for i in range(NJB)]
            aT = [persist.tile([128, T], BF16, tag=f"at{i}", name=f"at{i}")
                  for i in range(NET)]

            # v ones columns (softmax denominator accumulators) written
            # once; bv broadcast across partitions for C's fused bias add
            nc.gpsimd.partition_broadcast(bv_bc[:], bv_sb[:])
            for tt in range(NJB):
                vv = v_sb[tt][:].rearrange("p (h w) -> p h w", w=VH)
                nc.gpsimd.memset(vv[:, :, HD:VH], 1.0)

            def xq_c(d, t):
                h, o = t // 2, 512 * (t % 2)
                return xq_sb[(d, h)][:, o:o + 512]

            def xkv_c(d, t):
                h, o = t // 2, 512 * (t % 2)
                return xkv_sb[(d, h)][:, o:o + 512]

            # ---- filler bundle emitters (projections / output) ----
            def emit_A(et, t):
                ps = pps.tile([128, 512], F32, tag="pp")
                for d in range(NDT):
                    nc.tensor.matmul(
                        ps[:], wq_sb[d][:, 128 * et:128 * (et + 1)],
                        xq_c(d, t), start=(d == 0), stop=(d == NDT - 1))
                nc.vector.tensor_scalar_add(
                    qT[et][:, 512 * t:512 * (t + 1)], ps[:],
                    bq_sb[:, et:et + 1])

            def emit_B(et, t):
                ps = pps.tile([128, 512], F32, tag="pp")
                for d in range(NDT):
                    nc.tensor.matmul(
                        ps[:], wk_sb[d][:, 128 * et:128 * (et + 1)],
                        xkv_c(d, t), start=(d == 0), stop=(d == NDT - 1))
                nc.vector.tensor_scalar_add(
                    kT[et][:, 512 * t:512 * (t + 1)], ps[:],
                    bk_sb[:, et:et + 1])

            def emit_C(tt):
                ps = pps.tile([128, 512], F32, tag="pp")
                t, q = tt // 4, tt % 4
                for d in range(NDT):
                    nc.tensor.matmul(
                        ps[:], xkv_c(d, t)[:, 128 * q:128 * (q + 1)],
                        wv_sb[d][:], start=(d == 0), stop=(d == NDT - 1))
                vv = v_sb[tt][:].rearrange("p (h w) -> p h w", w=VH)
                nc.vector.tensor_tensor(
                    vv[:, :, 0:HD],
                    ps[:].rearrange("p (h w) -> p h w", w=HD),
                    bv_bc[:].rearrange("p (h w) -> p h w", w=HD),
                    mybir.AluOpType.add)

            def emit_E(do, ic):
                ps = pps.tile([128, 512], F32, tag="pp")
                for e in range(NET):
                    nc.tensor.matmul(
                        ps[:], wo_sb[e][:, 128 * do:128 * (do + 1)],
                        aT[e][:, 512 * ic:512 * (ic + 1)],
                        start=(e == 0), stop=(e == NET - 1))
                o = os_pool.tile([128, 512], BF16, tag="eo")
                nc.vector.tensor_scalar_add(o[:], ps[:], 0.0)
                nc.sync.dma_start(
                    outT[128 * do:128 * (do + 1), 512 * ic:512 * (ic + 1)],
                    o[:])

            def run_filler(f):
                kind = f[0]
                if kind == "A":
                    emit_A(f[1], f[2])
                elif kind == "B":
                    emit_B(f[1], f[2])
                elif kind == "C":
                    emit_C(f[1])
                else:
                    emit_E(f[1], f[2])

            # warmup: dummy matmuls on the first-arriving wq slab ramp
            # the PE p-state (full clock needs ~3us of continuous issue)
            # before the real prologue work, filling startup idle
            wu = pps.tile([128, 512], F32, tag="pp", name="warmup")
            for r in range(6):
                nc.tensor.matmul(wu[:], wq_sb[0][:, 0:128],
                                 wq_sb[0][:, 0:512],
                                 start=(r == 0), stop=(r == 5))
            # ---- prologue, ordered by DMA arrival (in-order PE queue:
            # an emission whose data is late stalls everything behind it).
            # wq+xq01 work first, then wk+xkv01, then wv (C's) last.
            for et in range(NET):
                emit_A(et, 0)
            for et in range(NET):
                emit_A(et, 1)
            for et in range(NET):
                emit_B(et, 0)
            for et in range(NET):
                emit_B(et, 1)
            for tt in range(5):
                emit_C(tt)

            # ---- phase D, ic-major, with interleaved fillers ----
            work = []   # (et, ic, jb, dlt, msk, first, last)
            for ic in range(NIC):
                lst = groups[ic]
                for et in range(NET):
                    for (jb, dlt, msk) in lst:
                        work.append((et, ic, jb, dlt, msk,
                                     jb == lst[0][0], jb == lst[-1][0]))

            # fillers per ic-phase, in first-need order.  All E emissions
            # are deferred to the ic3 phase: ic0..ic2 are PE-bound while
            # ic3 is ACT-bound (exp cols/block exceed PE stream cycles),
            # so E inventory there keeps the PE busy while ACT catches up.
            fillers = {
                # during D(.,0): v-blocks for ic1 and ic2 (ic0 is
                # slightly ACT-bound after the prologue restructure, so
                # give it the extra PE work; xkv23 lands in time for C9+)
                0: [("C", 5), ("C", 6), ("C", 7), ("C", 8),
                    ("C", 9), ("C", 10), ("C", 11), ("C", 12)],
                # during D(.,1): ic2 prep
                1: [("B", 0, 2), ("B", 1, 2), ("B", 2, 2), ("B", 3, 2),
                    ("A", 0, 2), ("A", 1, 2), ("A", 2, 2), ("A", 3, 2)],
                # during D(.,2): ic3 prep
                2: [("B", 0, 3), ("B", 1, 3), ("B", 2, 3), ("B", 3, 3),
                    ("A", 0, 3), ("A", 1, 3), ("A", 2, 3), ("A", 3, 3)]
                   + [("C", 13), ("C", 14), ("C", 15)],
                # during D(.,3): all ready E's
                3: [("E", do, 0) for do in range(NDT)]
                   + [("E", do, 1) for do in range(NDT)]
                   + [("E", do, 2) for do in range(NDT)],
            }

            ot = {}           # (et, ic) -> (otA, otB)
            pending = {}      # n -> pt tile
            queue = []
            DEPTH = 4

            def emit_div(et, ic, fast=False):
                otA, otB = ot.pop((et, ic))
                if fast:
                    # short-latency chain for the final group: skip the
                    # u-copies (mults read psum directly; nothing runs
                    # after, so psum residency doesn't matter)
                    ds = []
                    for o in (otA, otB):
                        dt = dv_pool.tile([1, 512], F32, tag="d")
                        nc.vector.tensor_scalar_add(dt[:], o[64:65, :], 0.0)
                        ds.append(dt)
                    rbs = []
                    for dt in ds:
                        r = dv_pool.tile([1, 512], F32, tag="r")
                        nc.vector.reciprocal_approx_fast(r[:], dt[:])
                        rb = dv_pool.tile([64, 512], F32, tag="rb")
                        nc.gpsimd.partition_broadcast(rb[:], r[:])
                        rbs.append(rb)
                    for half, (o, rb) in enumerate(zip((otA, otB), rbs)):
                        nc.vector.tensor_tensor(
                            aT[et][64 * half:64 * half + 64,
                                   512 * ic:512 * (ic + 1)],
                            o[0:64, :], rb[:], MUL)
                    return
                us, ds = [], []
                # two copies free the psum slot fast; the denominator row
                # goes to a base-partition-0 tile (reciprocal_approx_fast
                # mishandles nonzero base partitions)
                for o in (otA, otB):
                    u = dv_pool.tile([64, 512], F32, tag="u")
                    nc.vector.tensor_scalar_add(u[:], o[0:64, :], 0.0)
                    d = dv_pool.tile([1, 512], F32, tag="d")
                    nc.vector.tensor_scalar_add(d[:], o[64:65, :], 0.0)
                    us.append(u)
                    ds.append(d)
                rs = []
                for d in ds:
                    r = dv_pool.tile([1, 512], F32, tag="r")
                    nc.vector.reciprocal_approx_fast(r[:], d[:])
                    rs.append(r)
                rbs = []
                for r in rs:
                    rb = dv_pool.tile([64, 512], F32, tag="rb")
                    nc.gpsimd.partition_broadcast(rb[:], r[:])
                    rbs.append(rb)
                for half, (u, rb) in enumerate(zip(us, rbs)):
                    nc.vector.tensor_tensor(
                        aT[et][64 * half:64 * half + 64,
                               512 * ic:512 * (ic + 1)],
                        u[:], rb[:], MUL)

            def emit_stage2(n):
                et, ic, jb, dlt, msk, first, last = work[n]
                pt = pending.pop(n)
                if first:
                    ot[(et, ic)] = (
                        ops.tile([65, 512], F32, tag="ot", name=f"oA{et}_{ic}"),
                        ops.tile([65, 512], F32, tag="ot", name=f"oB{et}_{ic}"))
                otA, otB = ot[(et, ic)]
                hA, hB = 2 * et, 2 * et + 1
                nc.tensor.matmul(
                    otA[:, dlt:512], v_sb[jb][:, VH * hA:VH * hA + VH],
                    pt[:, dlt:512], start=first, stop=last,
                    skip_group_check=True)
                nc.tensor.matmul(
                    otB[:, dlt:512], v_sb[jb][:, VH * hB:VH * hB + VH],
                    pt[:, 512 + dlt:1024], start=first, stop=last,
                    skip_group_check=True)
                if last:
                    emit_div(et, ic, fast=True)

            # per-phase adaptive filler pacing
            ic_of = [w[1] for w in work]
            phase_len = [sum(1 for x in ic_of if x == ic) for ic in range(NIC)]
            acc = 0.0
            pos_in_phase = 0
            cur_ic = -1
            for n, (et, ic, jb, dlt, msk, first, last) in enumerate(work):
                if ic != cur_ic:
                    cur_ic = ic
                    pos_in_phase = 0
                    queue.extend(fillers[ic])
                g = pos_in_phase
                pos_in_phase += 1
                acc += len(queue) / max(1, phase_len[ic] - g)
                while acc >= 1.0 and queue:
                    run_filler(queue.pop(0))
                    acc -= 1.0

                st = sps.tile([128, 1024], F32, tag="st")
                nc.tensor.matmul(
                    st[:, dlt:512],
                    kT[et][0:64, 128 * jb:128 * (jb + 1)],
                    qT[et][0:64, 512 * ic + dlt:512 * (ic + 1)],
                    start=True, stop=True)
                nc.tensor.matmul(
                    st[:, 512 + dlt:1024],
                    kT[et][64:128, 128 * jb:128 * (jb + 1)],
                    qT[et][64:128, 512 * ic + dlt:512 * (ic + 1)],
                    start=True, stop=True)
                pt = pt_pool.tile([128, 1024], BF16, tag="pt")
                nc.scalar.activation(pt[:, dlt:1024], st[:, dlt:1024],
                                     AF.Exp, scale=SCALE)
                if msk:
                    k = dpos[128 * jb - L - 512 * ic]
                    w = min(dlt + 128, 512) - dlt
                    for off in (0, 512):
                        nc.vector.tensor_tensor(
                            pt[:, off + dlt:off + dlt + w],
                            pt[:, off + dlt:off + dlt + w],
                            mk_sb[:, 512 * k + dlt:512 * k + dlt + w], MUL)
                pending[n] = pt
                if n >= DEPTH:
                    emit_stage2(n - DEPTH)
            for n in range(max(0, len(work) - DEPTH), len(work)):
                emit_stage2(n)
            while queue:
                run_filler(queue.pop(0))
            # tail: E(.,3).  Four emissions pre-accumulate their e=0..2
            # matmuls (deps: aT[0..2] only) overlapping the final div
            # chain's latency -- two in pps slots, two borrowing the idle
            # sps (st) banks.  Tail output DMAs go out the scalar queue
            # (idle after the last exp) so the final drain isn't gated on
            # a single queue's packet rate.
            part = []
            for do in range(4):
                # sps slots first: they free as soon as the last exp is
                # read, while pps slots are held by the last E-fillers'
                # casts for ~1-2us more
                if do < 2:
                    ps = sps.tile([128, 1024], F32, tag="st",
                                  name=f"ep{do}")[:, 0:512]
                else:
                    ps = pps.tile([128, 512], F32, tag="pp",
                                  name=f"ep{do}")
                for e in range(NET - 1):
                    nc.tensor.matmul(
                        ps, wo_sb[e][:, 128 * do:128 * (do + 1)],
                        aT[e][:, 512 * 3:512 * 4],
                        start=(e == 0), stop=False)
                part.append(ps)
            for do in range(4):
                ps = part[do]
                nc.tensor.matmul(
                    ps, wo_sb[NET - 1][:, 128 * do:128 * (do + 1)],
                    aT[NET - 1][:, 512 * 3:512 * 4],
                    start=False, stop=True)
                o = os_pool.tile([128, 512], BF16, tag="eo")
                nc.vector.tensor_scalar_add(o[:], ps, 0.0)
                eng = nc.scalar if do % 2 == 0 else nc.sync
                eng.dma_start(
                    outT[128 * do:128 * (do + 1), 512 * 3:512 * 4], o[:])
            for do in range(4, NDT):
                ps = pps.tile([128, 512], F32, tag="pp")
                for e in range(NET):
                    nc.tensor.matmul(
                        ps[:], wo_sb[e][:, 128 * do:128 * (do + 1)],
                        aT[e][:, 512 * 3:512 * 4],
                        start=(e == 0), stop=(e == NET - 1))
                o = os_pool.tile([128, 512], BF16, tag="eo")
                nc.vector.tensor_scalar_add(o[:], ps[:], 0.0)
                eng = nc.scalar if do % 2 == 0 else nc.sync
                eng.dma_start(
                    outT[128 * do:128 * (do + 1), 512 * 3:512 * 4], o[:])

    nc.compile()
    return nc, deltas


def _prep_core(query, key_value, Wq, bq, Wk, bk, Wv, bv, Wo, c, deltas, L):
    b, half = c // 2, c % 2
    hs = E_LOC * half
    f32, bf16 = np.float32, ml_dtypes.bfloat16
    xqT = np.ascontiguousarray(query[b].T).astype(bf16)
    xkvT = np.ascontiguousarray(key_value[b].T).astype(bf16)
    wqT = np.ascontiguousarray(Wq[hs:hs + E_LOC].T).astype(bf16)
    wkT = np.ascontiguousarray(Wk[hs:hs + E_LOC].T).astype(bf16)
    wvT = np.ascontiguousarray(Wv[hs:hs + E_LOC].T).astype(bf16)
    bv_row = bv[hs:hs + E_LOC].reshape(1, E_LOC).astype(f32)
    woT = np.ascontiguousarray(Wo[:, hs:hs + E_LOC].T).astype(bf16)
    bq4 = np.ascontiguousarray(bq[hs:hs + E_LOC].reshape(NET, 128).T, dtype=f32)
    bk4 = np.ascontiguousarray(bk[hs:hs + E_LOC].reshape(NET, 128).T, dtype=f32)
    nmask = max(1, len(deltas))
    masks = np.zeros((128, nmask * 512), dtype=bf16)
    jr = np.arange(128)[:, None]
    ir = np.arange(512)[None, :]
    for k, d in enumerate(deltas):
        masks[:, 512 * k:512 * (k + 1)] = (jr <= ir - d).astype(bf16)
    ones2 = np.zeros((2, 128), dtype=f32)
    ones2[0, 0:64] = 1.0
    ones2[1, 64:128] = 1.0
    return {"xqT": xqT, "xkvT": xkvT, "wqT": wqT, "wkT": wkT, "wvT": wvT,
            "woT": woT, "bq4": bq4, "bk4": bk4, "bv_row": bv_row,
            "masks": masks, "ones2d": ones2}


def kernel(query, key_value, Wq, bq, Wk, bk, Wv, bv, Wo, bo, lookahead,
           _trace=False):
    L = int(lookahead)
    if L not in _CACHE:
        _CACHE[L] = _build(L)
    nc, deltas = _CACHE[L]

    args = [np.asarray(a, dtype=np.float32) for a in
            (query, key_value, Wq, bq, Wk, bk, Wv, bv, Wo)]
    in_maps = [_prep_core(*args, c, deltas, L) for c in range(8)]
    res = run_bass_kernel_spmd(nc, in_maps, core_ids=list(range(8)),
                               trace=_trace)
    bo = np.asarray(bo, dtype=np.float32)
    out = np.empty((B, T, D), dtype=np.float32)
    for b in range(B):
        pT = (res.results[2 * b]["outT"].astype(np.float32)
              + res.results[2 * b + 1]["outT"].astype(np.float32))
        out[b] = pT.T + bo[None, :]
    if _trace:
        kernel.last_exec_time_ns = res.exec_time_ns
    return out
